# revision 56
# baseline (speedup 1.0000x reference)
"""Fused attention kernel for TRN2, SPMD across 8 NeuronCores.

Problem: out = softmax(mask ? (Q Wq^T + bq)(K Wk^T + bk)^T / sqrt(D) : -1e9)
               @ (V Wv^T + bv)
with B=4, L=2048, E=D=1024.

Sharding: core c handles batch b=c//2, query-half h=c%2 (1024 query rows).
No collectives; K/V rows for the batch are fully loaded per core.

Algebra (per core; Xq = Q-shard (1024,E), Xk = K[b] (2048,E), Xv = V[b]):
  tT     = (Wqk^T Xq^T + kb 1^T) * 32          Wqk = Wq^T Wk, kb = Wk^T bq
  sT     = Xk tT                               (scores TRANSPOSED: [J, L])
  pT     = exp(sT/1024) * mT                   (mask multiplied post-exp)
  zT     = Xv^T pT (fp8 hi/lo 3-term)          [D, L] per pair, x1/16
  out    = (zT^T WvT) * 1/(4*denom) + 1 (x) bv  (WvT carries x64)
  denom  = 1^T pTh   -- a PE matmul with stationary pmH and a ones moving
           vector: out [L-part, 1] costs ~0 cycles (cost ~ out free size)
           and lands in the per-partition layout P5's scale needs.

All matmuls are fp8 e4m3 DoubleRow (0.5 cyc/row):
  P1 (Q proj):  single-fp8 Wqk x single-fp8 Xq -> tT fp8 (+kb bias);
                tT is consumed as fp8 anyway, so 1-term is enough.
  P2 (scores):  computed transposed, so softmax emits pT directly and no
                PE transposes are needed.
  P4/P5:        3-term hi/lo (drop lo*lo), as in the x64/x16 scale scheme.

Scheduling notes (cost-model-driven):
 - Softmax chain per [P,1024] psum tile: exp (ACT) -> pm = pexp*mT
   (DVE, all-bf16 so the 2x mode applies -- any fp8 operand halves DVE
   throughput) -> hi = fp8(pm) (alternating ACT/DVE) -> lo = pm - hi
   (DVE). Mask is sent as bf16 for exactly this reason.
 - Pipeline order per pair: P4(k); P5(k); head(k+2). P5's psum
   evictions must enqueue on DVE BEFORE the next head's chain ops, or
   P4(k+1) stalls on its psum-bank WAW behind ~8us of chain work.
 - P4 halves map to contiguous Vb column halves via a zT chunk
   permutation folded into host-side WvT, so Vb streams as two 2MB
   column-blocks; jp-outer within. dn(pair) rides mid-P4(pair).
 - All loads on the SP queue in strict consumption order (transfers
   serialize across queues in this machine; issue order is priority).
   XkT streams as contiguous J-halves matching the g-major head loop.
 - A PSUM bank supports ONE open accumulation group: groups sharing a
   bank are always emitted stop-before-start (interleaving corrupts
   results on hardware while passing the simulator).
"""
from contextlib import ExitStack

import numpy as np

import concourse.bacc as bacc
import concourse.tile as tile
from concourse import mybir
from concourse.bass_utils import run_bass_kernel_spmd
from concourse.masks import make_identity

F32 = mybir.dt.float32
BF16 = mybir.dt.bfloat16
FP8 = mybir.dt.float8e4
AF = mybir.ActivationFunctionType
ALU = mybir.AluOpType
DR = mybir.MatmulPerfMode.DoubleRow

B, L, E, D = 4, 2048, 1024, 1024
LS = 1024          # query rows per core
J = 2048           # key rows per core
P = 128
NCORES = 8
WQK_SCALE = 32.0   # host-side Wqk/kb scale keeps fp8 hi/lo in normal range
SCALE = 1.0 / 32.0 / WQK_SCALE  # 1/sqrt(D)/WQK_SCALE, applied at the Exp

EC = E // P        # 8 chunks of 128 along E/D dims
JC = J // P        # 16 chunks along J
LT = LS // P       # 8 query tiles per core
NP = LT // 2       # 4 query-tile pairs


def _r8(ap):
    # [P, 1024] -> [P, 8, 128] view so free dims match strided pT slices
    return ap.rearrange("p (c q) -> p c q", c=8)


def _build():
    nc = bacc.Bacc(None, target_bir_lowering=False)

    Xq_e = nc.declare_dram_parameter("XqT", [E, LS], FP8, isOutput=False)
    Xk_e = nc.declare_dram_parameter("XkT", [2, E, J // 2], FP8,
                                     isOutput=False)
    VbH_e = nc.declare_dram_parameter("VbH", [2, J, E // 2], FP8,
                                      isOutput=False)
    VbL_e = nc.declare_dram_parameter("VbL", [2, J, E // 2], FP8,
                                      isOutput=False)
    MT_e = nc.declare_dram_parameter("mT", [LT, J, P], BF16,
                                     isOutput=False)
    WqkH_e = nc.declare_dram_parameter("WqkH", [E, E], FP8, isOutput=False)
    kb_e = nc.declare_dram_parameter("kb", [E], F32, isOutput=False)
    WvH_e = nc.declare_dram_parameter("WvH", [E, D], FP8, isOutput=False)
    WvL_e = nc.declare_dram_parameter("WvL", [E, D], FP8, isOutput=False)
    bv_e = nc.declare_dram_parameter("bv", [D], F32, isOutput=False)
    out_e = nc.declare_dram_parameter("out", [LS, D], F32, isOutput=True)

    XqT_d = Xq_e.ap().rearrange("(c p) l -> p c l", p=P)
    XkT_d = Xk_e.ap().rearrange("h (c p) j -> p h c j", p=P)
    VbH_d = VbH_e.ap().rearrange("h (c p) e -> p h c e", p=P)
    VbL_d = VbL_e.ap().rearrange("h (c p) e -> p h c e", p=P)
    MT_d = MT_e.ap().rearrange("t (c p) q -> p t c q", p=P)
    WqkH_d = WqkH_e.ap().rearrange("(c p) e -> p c e", p=P)
    kb_d = kb_e.ap().rearrange("(c p) -> p c", p=P)
    WvH_d = WvH_e.ap().rearrange("(c p) d -> p c d", p=P)
    WvL_d = WvL_e.ap().rearrange("(c p) d -> p c d", p=P)
    out_d = out_e.ap().rearrange("(c p) d -> p c d", p=P)

    with tile.TileContext(nc) as tc, ExitStack() as long_pools:
        lp_pool = lambda name, bufs=1: long_pools.enter_context(
            tc.tile_pool(name=name, bufs=bufs))
        # ---- constants ----
        consts = lp_pool("consts")
        ident_f = consts.tile([P, P], F32, name="ident_f")
        make_identity(nc, ident_f[:])
        ident_b = consts.tile([P, P], BF16, name="ident_b")
        nc.vector.tensor_copy(ident_b[:], ident_f[:])
        bvb_sb = consts.tile([P, D], F32, name="bvb_sb")
        kb_sb = consts.tile([P, EC], F32, name="kb_sb")
        onesJ = consts.tile([P, JC, 1], FP8, name="onesJ")
        nc.vector.memset(onesJ[:], 1.0)

        tT_sb = lp_pool("tT_p").tile([P, EC, LS], FP8, name="tT_sb")
        XkT_sb = lp_pool("XkT_p").tile([P, EC, J], FP8, name="XkT_sb")
        mT_sb = lp_pool("mT_p").tile([P, LT, JC, P], BF16, name="mT_sb")
        VbH_sb = lp_pool("VbH_p").tile([P, JC, D], FP8, name="VbH_sb")
        VbL_sb = lp_pool("VbL_p").tile([P, JC, D], FP8, name="VbL_sb")
        WvH_sb = lp_pool("WvH_p").tile([P, EC, D], FP8, name="WvH_sb")
        WvL_sb = lp_pool("WvL_p").tile([P, EC, D], FP8, name="WvL_sb")

        ptp = lp_pool("ptp", bufs=3)    # pTh/pTl per pair [P, JC, 2P]
        pxp = lp_pool("pxp", bufs=3)    # exp staging [P, 8, P] bf16
        pmp = lp_pool("pmp", bufs=3)    # masked-p staging [P, 8, P] bf16
        dnp = lp_pool("dn", bufs=2)     # den4/rden [P, 1] f32 (per lh tag)
        ztp = lp_pool("ztp", bufs=2)    # zT hi/lo [P, EC, 2P] fp8
        opool = lp_pool("op", bufs=3)   # o_sb [P, D] f32

        # ======== stage A: loads (SP queue) + P1 (own psum pool) =========
        with (
            tc.tile_pool(name="ps_p1", bufs=1, space="PSUM") as ps_p1,
            tc.tile_pool(name="wqk_pool", bufs=1) as wqk_pool,
        ):
            wqkH_sb = wqk_pool.tile([P, EC, E], FP8, name="wqkH_sb")
            xqT_sb = wqk_pool.tile([P, EC, LS], FP8, name="xqT_sb")
            # P1 first (pair-wise), then XkT by J-halves (the g-major
            # head loop consumes j-half 0 for both lh before j-half 1)
            for pc in range(EC // 2):
                s = slice(2 * pc, 2 * pc + 2)
                nc.sync.dma_start(out=wqkH_sb[:, s, :], in_=WqkH_d[:, s, :])
                nc.sync.dma_start(out=xqT_sb[:, s, :], in_=XqT_d[:, s, :])
                if pc == 0:
                    nc.sync.dma_start(out=kb_sb[:], in_=kb_d)
            nc.sync.dma_start(out=XkT_sb[:, :, 0:1024], in_=XkT_d[:, 0])
            nc.sync.dma_start(out=XkT_sb[:, :, 1024:2048], in_=XkT_d[:, 1])
            nc.sync.dma_start(out=mT_sb[:, 0:2], in_=MT_d[:, 0:2])
            nc.sync.dma_start(out=mT_sb[:, 2:4], in_=MT_d[:, 2:4])
            import concourse.bass as _bass
            bv_bcast = _bass.AP(tensor=bv_e, offset=0, ap=[[0, P], [1, D]])
            nc.sync.dma_start(out=bvb_sb[:], in_=bv_bcast)
            # Vb by E-column halves in P4 half order, jp-paced within
            for jp in range(4):
                s = slice(4 * jp, 4 * jp + 4)
                nc.sync.dma_start(out=VbH_sb[:, s, 0:512],
                                  in_=VbH_d[:, 0, s, :])
                nc.sync.dma_start(out=VbL_sb[:, s, 0:512],
                                  in_=VbL_d[:, 0, s, :])
            for jp in range(4):
                s = slice(4 * jp, 4 * jp + 4)
                nc.sync.dma_start(out=VbH_sb[:, s, 512:1024],
                                  in_=VbH_d[:, 1, s, :])
                nc.sync.dma_start(out=VbL_sb[:, s, 512:1024],
                                  in_=VbL_d[:, 1, s, :])
            nc.sync.dma_start(out=mT_sb[:, 4:6], in_=MT_d[:, 4:6])
            nc.sync.dma_start(out=mT_sb[:, 6:8], in_=MT_d[:, 6:8])
            # Wv per dp-pair (P5 dp order)
            for dp in range(4):
                s = slice(2 * dp, 2 * dp + 2)
                nc.sync.dma_start(out=WvH_sb[:, s, :], in_=WvH_d[:, s, :])
                nc.sync.dma_start(out=WvL_sb[:, s, :], in_=WvL_d[:, s, :])

            # PE warmup out of the phase-1 psum banks (WAW with the P1
            # groups just orders them on the in-order PE)
            for wu in range(6):
                ps = ps_p1.tile([P, 512], BF16, name="pswu",
                                tag=f"p1_{wu % 4}")
                for k in range(4):
                    nc.tensor.transpose(ps[:, k * P:(k + 1) * P],
                                        ident_b[:], ident_b[:])

            def emit_p1_sub(e2ts):
                # [P,1024] psum per e2t (2 banks: one per l-half), one
                # wide eviction per e2t with the kb bias riding it
                pss = {}
                for i, e2t in enumerate(e2ts):
                    pss[e2t] = ps_p1.tile([P, 1024], F32,
                                          name=f"ps1_{e2t}",
                                          tag=f"p1_{e2t % 4}")
                for pc in range(EC // 2):
                    s = slice(2 * pc, 2 * pc + 2)
                    for e2t in e2ts:
                        for lc in range(2):
                            nc.tensor.matmul(
                                pss[e2t][:, lc * 512:(lc + 1) * 512],
                                wqkH_sb[:, s, e2t * P:(e2t + 1) * P],
                                xqT_sb[:, s, lc * 512:(lc + 1) * 512],
                                start=(pc == 0),
                                stop=(pc == EC // 2 - 1),
                                perf_mode=DR,
                            )
                for i, e2t in enumerate(e2ts):
                    dst = tT_sb[:, e2t, :]
                    if i % 2 == 0:
                        nc.scalar.activation(
                            out=dst, in_=pss[e2t][:], func=AF.Identity,
                            bias=kb_sb[:, e2t:e2t + 1],
                        )
                    else:
                        nc.vector.tensor_scalar(
                            out=dst, in0=pss[e2t][:],
                            scalar1=kb_sb[:, e2t:e2t + 1],
                            scalar2=None, op0=ALU.add,
                        )

            emit_p1_sub(range(0, 4))
            emit_p1_sub(range(4, 6))
            emit_p1_sub(range(6, 8))

        # ======== stage B: heads + P4/P5 pair pipeline ===================
        with (
            tc.tile_pool(name="ps_sc", bufs=1, space="PSUM") as ps_sc,
            tc.tile_pool(name="ps_mm", bufs=1, space="PSUM") as ps_mm,
        ):
            def emit_head(pair, rides=None, lhs=(0, 1), into=None):
                # scores (transposed) + softmax for lts (2*pair, 2*pair+1)
                if into is None:
                    pTh = ptp.tile([P, JC, 2 * P], FP8, name="pTh", tag="th")
                    pTl = ptp.tile([P, JC, 2 * P], FP8, name="pTl", tag="tl")
                else:
                    pTh, pTl = into
                for g in range(2):
                    for lh in lhs:
                        lt = 2 * pair + lh
                        ps = ps_sc.tile([P, 1024], F32, name="ps_sc",
                                        tag=f"sc{lh if len(lhs) > 1 else g}")
                        for jj in range(8):
                            jc = 8 * g + jj
                            for ep in range(4):
                                nc.tensor.matmul(
                                    ps[:, jj * P:(jj + 1) * P],
                                    XkT_sb[:, 2 * ep:2 * ep + 2,
                                           jc * P:(jc + 1) * P],
                                    tT_sb[:, 2 * ep:2 * ep + 2,
                                          lt * P:(lt + 1) * P],
                                    start=(ep == 0), stop=(ep == 3),
                                    perf_mode=DR,
                                )
                        pexp = pxp.tile([P, 8, P], BF16, name="pexp",
                                        tag="px")
                        nc.scalar.activation(out=pexp[:], in_=_r8(ps[:]),
                                             func=AF.Exp, scale=SCALE)
                        mTg = mT_sb[:, lt, 8 * g:8 * g + 8, :]
                        # pm all-bf16 so DVE runs in 2x mode; the fp8 hi
                        # conversion alternates ACT/DVE to balance queues
                        pm = pmp.tile([P, 8, P], BF16, name="pm", tag="pm")
                        nc.vector.tensor_tensor(
                            out=pm[:], in0=pexp[:], in1=mTg, op=ALU.mult,
                        )
                        hi = pTh[:, 8 * g:8 * g + 8, lh * P:(lh + 1) * P]
                        if (2 * lh + g) % 2 == 0:
                            nc.scalar.activation(out=hi, in_=pm[:],
                                                 func=AF.Copy)
                        else:
                            nc.vector.tensor_copy(hi, pm[:])
                        nc.vector.tensor_tensor(
                            out=pTl[:, 8 * g:8 * g + 8,
                                    lh * P:(lh + 1) * P],
                            in0=pm[:], in1=hi, op=ALU.subtract,
                        )
                        if g == 1 and rides is not None and lh in rides:
                            rides[lh]()
                return pTh, pTl

            def emit_dn(pair, pTh):
                # denom via stationary-pmH x ones matmuls: out [L-part, 1]
                dn = ps_sc.tile([P, 2], F32, name="dn", tag="sc0")
                for lh in range(2):
                    for jp in range(JC // 2):
                        nc.tensor.matmul(
                            dn[:, lh:lh + 1],
                            pTh[:, 2 * jp:2 * jp + 2, lh * P:(lh + 1) * P],
                            onesJ[:, 2 * jp:2 * jp + 2, :],
                            start=(jp == 0), stop=(jp == JC // 2 - 1),
                            perf_mode=DR,
                        )
                rdens = []
                for lh in range(2):
                    den4 = dnp.tile([P, 1], F32, name="den4", tag=f"d4{lh}")
                    nc.vector.tensor_scalar(out=den4[:],
                                            in0=dn[:, lh:lh + 1],
                                            scalar1=4.0, scalar2=None,
                                            op0=ALU.mult)
                    rden = dnp.tile([P, 1], F32, name="rden", tag=f"rd{lh}")
                    nc.vector.reciprocal(out=rden[:], in_=den4[:])
                    rdens.append(rden)
                return rdens

            def emit_p4(pair, pT, ride=None):
                # zT[dt] = Xv^T pT, fp8 3-term; two dt-rounds of 2 banks
                # (round r = dt 4r..4r+3 = Vb columns r*512:(r+1)*512, so
                # Vb streams by column halves); jp-outer within a round;
                # eviction per bank as fp8 hi/lo /16, spread mid-P4
                pTh, pTl = pT
                zTh = ztp.tile([P, EC, 2 * P], FP8, name="zTh", tag="zh")
                zTl = ztp.tile([P, EC, 2 * P], FP8, name="zTl", tag="zl")
                terms = [(pTh, VbH_sb), (pTh, VbL_sb), (pTl, VbH_sb)]
                pss = [ps_mm.tile([P, 512], F32, name=f"ps4_{i}",
                                  tag=f"mm{i}") for i in range(4)]
                for half in range(2):
                    for jp in range(JC // 2):
                        for ti, (pt, vb) in enumerate(terms):
                            for dtp in range(4):
                                dt = dtp + 4 * half
                                nc.tensor.matmul(
                                    pss[dtp][:, half * 256:half * 256 + 256],
                                    vb[:, 2 * jp:2 * jp + 2,
                                       dt * P:(dt + 1) * P],
                                    pt[:, 2 * jp:2 * jp + 2, :],
                                    start=(jp == 0 and ti == 0),
                                    stop=(jp == JC // 2 - 1 and ti == 2),
                                    perf_mode=DR,
                                )
                    if ride is not None and half == 0:
                        ride()
                for dtp in range(4):
                    sp = pss[dtp][:].rearrange("p (a q) -> p a q", a=2)
                    hi = zTh[:, 2 * dtp:2 * dtp + 2, :]
                    nc.scalar.activation(out=hi, in_=sp,
                                         func=AF.Copy, scale=1.0 / 16.0)
                    nc.vector.scalar_tensor_tensor(
                        out=zTl[:, 2 * dtp:2 * dtp + 2, :],
                        in0=sp, scalar=1.0 / 16.0, in1=hi,
                        op0=ALU.mult, op1=ALU.subtract,
                    )
                return zTh, zTl

            def emit_p5_group(pair, zT, rdens, lh, doc, o_sb, pieces):
                # out[:, doc-half] = (zT^T WvT)*rden + bv; dp-outer.
                # pieces: list of (c0, c1, eng) eviction/store splits.
                zTh, zTl = zT
                lt = 2 * pair + lh
                terms = [(zTh, WvH_sb), (zTh, WvL_sb), (zTl, WvH_sb)]
                ps = ps_mm.tile([P, 512], F32, name="ps5",
                                tag=f"mm{2 * lh + doc}")
                n = 0
                for dp in range(EC // 2):
                    for zt, wv in terms:
                        nc.tensor.matmul(
                            ps[:],
                            zt[:, 2 * dp:2 * dp + 2, lh * P:(lh + 1) * P],
                            wv[:, 2 * dp:2 * dp + 2,
                               doc * 512:(doc + 1) * 512],
                            start=(n == 0), stop=(n == 3 * EC // 2 - 1),
                            perf_mode=DR,
                        )
                        n += 1
                for pi, (c0, c1, eng) in enumerate(pieces):
                    ev = nc.vector
                    ev.scalar_tensor_tensor(
                        out=o_sb[:, doc * 512 + c0:doc * 512 + c1],
                        in0=ps[:, c0:c1], scalar=rdens[lh][:],
                        in1=bvb_sb[:, doc * 512 + c0:doc * 512 + c1],
                        op0=ALU.mult, op1=ALU.add,
                    )
                    eng.dma_start(
                        out=out_d[:, lt, doc * 512 + c0:doc * 512 + c1],
                        in_=o_sb[:, doc * 512 + c0:doc * 512 + c1])

            # ===== pair pipeline =====
            pT = {0: emit_head(0)}
            rd = {}

            def dn_ride(q):
                def r():
                    rd[q] = emit_dn(q, pT[q][0])
                return r

            pT[1] = emit_head(1)

            for pair in range(NP):
                last = pair == NP - 1
                # dn(pair) rides mid-P4(pair): pmH(pair)'s chain finished
                # at least a pair earlier, and rden is needed by P5(pair)
                zT = emit_p4(pair, pT[pair], ride=dn_ride(pair))
                o_sbs = {lh: opool.tile([P, D], F32, name="o_sb", tag="o",
                                        bufs=4) for lh in range(2)}
                for lh in range(2):
                    for doc in range(2):
                        if not last:
                            pieces = [(0, 512, nc.sync)]
                        else:
                            pieces = [(0, 256, nc.sync),
                                      (256, 512, nc.scalar)]
                        emit_p5_group(pair, zT, rd[pair], lh, doc,
                                      o_sbs[lh], pieces)
                if pair == 0:
                    pT[2] = emit_head(2)
                elif pair == 1:
                    pT[3] = emit_head(3, lhs=(0,))
                elif pair == 2:
                    emit_head(3, lhs=(1,), into=pT[3])

    nc.compile()
    return nc


_NC_CACHE = {}


def _get_nc():
    if "nc" not in _NC_CACHE:
        _NC_CACHE["nc"] = _build()
    return _NC_CACHE["nc"]


def _shard_inputs(Q, K, V, mask, Wq_w, Wq_b, Wk_w, Wk_b, Wv_w, Wv_b):
    import ml_dtypes
    fp8 = ml_dtypes.float8_e4m3
    f32 = np.float32

    def hilo(x):
        hi = x.astype(fp8)
        lo = (x - hi.astype(f32)).astype(fp8)
        return (np.ascontiguousarray(hi), np.ascontiguousarray(lo))

    Wq32 = np.asarray(Wq_w, f32)
    Wk32 = np.asarray(Wk_w, f32)
    # Score scale 1/sqrt(D) and the x32 Wqk normalization are both applied
    # at the Exp activation (scale=1/1024). WvT is scaled x64 so its fp8
    # hi/lo escapes the subnormal floor; z carries 1/16 — cancelled by the
    # final 1/(4*denom) row scale.
    WvT = np.asarray(Wv_w, f32).T * 64.0
    perm = [0, 4, 1, 5, 2, 6, 3, 7]
    WvT = WvT.reshape(8, 128, -1)[perm].reshape(1024, -1)
    WvH, WvL = hilo(WvT)
    WqkH = np.ascontiguousarray(
        ((Wq32.T @ Wk32) * WQK_SCALE).astype(fp8))
    common = {
        "WqkH": WqkH,
        "kb": np.ascontiguousarray(
            (Wk32.T @ np.asarray(Wq_b, f32)) * WQK_SCALE, f32),
        "WvH": WvH, "WvL": WvL,
        "bv": np.ascontiguousarray(Wv_b, f32),
    }
    in_maps = []
    for c in range(NCORES):
        b, h = divmod(c, 2)
        sl = slice(h * LS, (h + 1) * LS)
        Vb2 = np.asarray(V[b], f32).reshape(J, 2, E // 2)
        Vb2 = np.ascontiguousarray(Vb2.transpose(1, 0, 2))
        VbH, VbL = hilo(Vb2)
        # mT: per-lt blocks of the transposed mask, [LT, J, P]
        import ml_dtypes as _md
        mT = np.asarray(mask[b, sl, :]).T.astype(_md.bfloat16)  # [J, LS]
        mT = np.ascontiguousarray(
            mT.reshape(J, LT, P).transpose(1, 0, 2))
        in_maps.append({
            "XqT": np.ascontiguousarray(
                np.asarray(Q[b, sl, :], f32).T.astype(fp8)),
            "XkT": np.ascontiguousarray(
                np.asarray(K[b], f32).astype(fp8).T.reshape(
                    E, 2, J // 2).transpose(1, 0, 2)),
            "VbH": VbH, "VbL": VbL,
            "mT": mT,
            **common,
        })
    return in_maps


def _run(inputs, trace=False):
    nc = _get_nc()
    in_maps = _shard_inputs(**inputs)
    res = run_bass_kernel_spmd(nc, in_maps, core_ids=list(range(NCORES)),
                               trace=trace)
    out = np.empty((B, L, D), np.float32)
    for c in range(NCORES):
        b, h = divmod(c, 2)
        out[b, h * LS:(h + 1) * LS, :] = res.results[c]["out"]
    return out, res


def kernel(**inputs):
    out, _ = _run(inputs, trace=False)
    return out
